# revision 39
# baseline (speedup 1.0000x reference)
"""Trainium2 Bass kernel for nn_BranchRoute (threshold MoE routing).

reference:
    score = sigmoid(x @ W_gate + b_gate)          # [N, 2]
    hot   = score > 0.5                           # == (x @ W_gate + b_gate) > 0
    x_0   = where(hot[:, 0:1], x, 0)
    x_1   = where(hot[:, 1:2], x, 0)
    x_comb = x_0 + x_1

Sharding: data-parallel over tokens across 8 NeuronCores (2048 tokens/core),
gate weights replicated.  Per core the kernel streams 16 tiles of
[128 tokens, 1024 d]: gate logits via fused multiply+reduce on DVE
(sigmoid(z) > 0.5  <=>  z > -b, so no sigmoid is evaluated), then three
per-partition-scalar mask multiplies (x0/x1 on ACT, x_comb on DVE), and
per-tile stores split across both HWDGE queues while loads prefetch on the
Pool SWDGE queue.  The kernel is DMA-bound: 8 MiB in + 24 MiB out per core
~= 93 us at the measured ~368 GB/s per-core HBM rate; it executes in
~99-105 us.
"""

import numpy as np

N_TOKENS = 16384
D_MODEL = 1024
N_BRANCHES = 2
N_CORES = 8
N_SHARD = N_TOKENS // N_CORES  # 2048 tokens per core
P = 128                        # SBUF partitions
NTILES = N_SHARD // P          # 16 token-tiles per core

_CACHE = {}


def _split_multi_waits(nc, max_embedded=1):
    """This container's walrus build rejects instructions carrying more than
    one embedded semaphore wait ("Too many sync wait commands").  Hoist the
    extra waits into standalone EventSemaphore instructions immediately
    before the owning instruction on the same engine — identical ordering
    semantics, encodable by this compiler."""
    from concourse import mybir

    wid = 0
    for fn in nc.m.functions:
        for bb in fn.blocks:
            out = []
            changed = False
            for inst in bb.instructions:
                si = getattr(inst, "sync_info", None)
                waits = list(si.on_wait) if si is not None else []
                if si is not None and len(waits) > max_embedded:
                    extra, keep = waits[:-max_embedded], waits[-max_embedded:]
                    for w in extra:
                        es = mybir.InstEventSemaphore(
                            name=f"WSPLIT-{wid}", ins=[], outs=[]
                        )
                        wid += 1
                        es.engine = inst.engine
                        es.sync_info = mybir.SyncInfo(on_wait=[w], on_update=[])
                        out.append(es)
                    si.on_wait = keep
                    changed = True
                out.append(inst)
            if changed:
                bb.instructions = out


def _build_bass(dma_cfg="C"):
    import concourse.bass as bass
    import concourse.tile as tile
    from concourse import mybir

    f32 = mybir.dt.float32
    nc = bass.Bass(trn_type="TRN2", num_swdge_queues=2)

    # w is passed host-side as [N_BRANCHES, D_MODEL + 1]: row br holds
    # W[:, br] transposed (contiguous, so the partition-broadcast DMA reads
    # 4 KiB bursts) with -b[br] appended as the last column.
    DW = D_MODEL + 1
    x_h = nc.dram_tensor("x", [N_SHARD, D_MODEL], f32, kind="ExternalInput")
    w_h = nc.dram_tensor("w", [N_BRANCHES, DW], f32, kind="ExternalInput")
    o0_h = nc.dram_tensor("o0", [N_SHARD, D_MODEL], f32, kind="ExternalOutput")
    o1_h = nc.dram_tensor("o1", [N_SHARD, D_MODEL], f32, kind="ExternalOutput")
    oc_h = nc.dram_tensor("oc", [N_SHARD, D_MODEL], f32, kind="ExternalOutput")

    # Pair token-tiles: [npair, 128, 2, 1024] — one 1 MiB DMA per pair,
    # partition dim leading on both sides so the DMA APs balance.
    TB = 2
    NPAIR = NTILES // TB
    x_t = x_h[:].rearrange("(t s p) d -> t p s d", s=TB, p=P)
    o0_t = o0_h[:].rearrange("(t s p) d -> t p s d", s=TB, p=P)
    o1_t = o1_h[:].rearrange("(t s p) d -> t p s d", s=TB, p=P)
    oc_t = oc_h[:].rearrange("(t s p) d -> t p s d", s=TB, p=P)

    with tile.TileContext(nc) as tc:
        with (
            tc.tile_pool(name="singles", bufs=1) as singles,
            tc.tile_pool(name="xp", bufs=6) as xp,
            tc.tile_pool(name="scr", bufs=3) as scr,
            tc.tile_pool(name="out0", bufs=4) as p0,
            tc.tile_pool(name="out1", bufs=4) as p1,
            tc.tile_pool(name="outc", bufs=4) as pc,
            tc.tile_pool(name="small", bufs=8) as small,
        ):
            # [W^T | -b] rows broadcast across all 128 partitions.  A single
            # step-0-partition DRAM broadcast DMA measures ~10us and stalls
            # startup, so split it into 4 concurrent 32-partition chunks on
            # the ACT HWDGE queue (the SP queue carries the first x load).
            # (A serial on-chip doubling chain was measured slower: each
            # chained SBUF->SBUF DMA pays issue+sem latency.)
            # wb[p, br*DW : br*DW+D] = W[:, br],  wb[p, br*DW+D] = -b[br]
            wb = singles.tile([P, N_BRANCHES * DW], f32)
            w_ap = w_h[:]
            PCHUNK = 32
            for ci in range(P // PCHUNK):
                w_bcast = bass.AP(
                    tensor=w_ap.tensor,
                    offset=w_ap.offset,
                    ap=[[0, PCHUNK], [1, N_BRANCHES * DW]],
                )
                nc.scalar.dma_start(
                    out=wb[ci * PCHUNK : (ci + 1) * PCHUNK, :], in_=w_bcast
                )
            # negb[p, br] = -b[br] as a strided view of wb
            negb = bass.AP(
                tensor=wb.tensor,
                offset=wb.offset + D_MODEL,
                ap=[wb.ap[0], [DW, N_BRANCHES]],
            )

            for i in range(NPAIR):
                # dma_cfg picks the load queue: A = ACT HWDGE, B = SP HWDGE,
                # C = Pool SWDGE (prefetched ahead, so SWDGE latency hides)
                # except the critical first pair, which takes the fast SP
                # HWDGE path to cut the startup stall.
                x_sb = xp.tile([P, TB, D_MODEL], f32)
                ld = {
                    "A": nc.scalar,
                    "B": nc.sync,
                    "C": nc.sync if i == 0 else nc.gpsimd,
                }[dma_cfg]
                ld.dma_start(out=x_sb, in_=x_t[i])

                op0_pair = op1_pair = opc_pair = None
                if dma_cfg in ("A", "B"):
                    op0_pair = p0.tile([P, TB, D_MODEL], f32, tag="o0pair")
                    op1_pair = p1.tile([P, TB, D_MODEL], f32, tag="o1pair")
                    opc_pair = pc.tile([P, TB, D_MODEL], f32, tag="ocpair")

                for s in range(TB):
                    x_s = x_sb[:, s, :]

                    # z[p, br] = sum_d x[p, d] * W[d, br]  (fused DVE pass/branch)
                    z = small.tile([P, N_BRANCHES], f32)
                    for br in range(N_BRANCHES):
                        scratch = scr.tile([P, D_MODEL], f32)
                        nc.vector.scalar_tensor_tensor(
                            out=scratch,
                            in0=x_s,
                            scalar=0.0,
                            in1=wb[:, br * DW : br * DW + D_MODEL],
                            op0=mybir.AluOpType.bypass,
                            op1=mybir.AluOpType.mult,
                            accum_out=z[:, br : br + 1],
                        )

                    # hot mask: m = (z > -b) as 1.0/0.0 ; mc = m0 + m1
                    m = small.tile([P, N_BRANCHES], f32)
                    nc.vector.tensor_tensor(
                        out=m, in0=z, in1=negb, op=mybir.AluOpType.is_gt
                    )
                    mc = small.tile([P, 1], f32)
                    nc.vector.tensor_add(out=mc, in0=m[:, 0:1], in1=m[:, 1:2])

                    # masked outputs: x * m (per-partition scalar broadcast)
                    if dma_cfg == "C":
                        o0 = p0.tile([P, D_MODEL], f32)
                        o1 = p1.tile([P, D_MODEL], f32)
                        oc = pc.tile([P, D_MODEL], f32)
                    else:
                        o0 = op0_pair[:, s, :]
                        o1 = op1_pair[:, s, :]
                        oc = opc_pair[:, s, :]
                    nc.scalar.mul(out=o0, in_=x_s, mul=m[:, 0:1])
                    nc.scalar.mul(out=o1, in_=x_s, mul=m[:, 1:2])
                    nc.vector.tensor_scalar_mul(out=oc, in0=x_s, scalar1=mc)

                    if dma_cfg == "C":
                        # Per-sub-tile stores split across both HWDGE queues
                        # (12 MiB per queue).
                        qa = nc.sync if (i * TB + s) % 2 == 0 else nc.scalar
                        nc.sync.dma_start(out=o0_t[i][:, s, :], in_=o0)
                        nc.scalar.dma_start(out=o1_t[i][:, s, :], in_=o1)
                        qa.dma_start(out=oc_t[i][:, s, :], in_=oc)

                if dma_cfg == "A":
                    # Loads on ACT queue; all stores on the SP queue.
                    nc.sync.dma_start(out=o0_t[i], in_=op0_pair)
                    nc.sync.dma_start(out=o1_t[i], in_=op1_pair)
                    nc.sync.dma_start(out=oc_t[i], in_=opc_pair)
                elif dma_cfg == "B":
                    # Loads + oc on SP queue; o0/o1 on the ACT queue.
                    nc.scalar.dma_start(out=o0_t[i], in_=op0_pair)
                    nc.scalar.dma_start(out=o1_t[i], in_=op1_pair)
                    nc.sync.dma_start(out=oc_t[i], in_=opc_pair)

    _split_multi_waits(nc)
    return nc


def _get_nc():
    if "nc" not in _CACHE:
        _CACHE["nc"] = _build_bass()
    return _CACHE["nc"]


LAST_EXEC_NS = None
LAST_TRACE = None


def _ensure_ntff_shim():
    """antenv.axon_hooks is absent in this container image; when tracing is
    active (trace=True or BASS_TRACE set) run_bass_kernel_spmd imports it.
    Recreate it from the ctypes implementation shipped in trn_agent_boot."""
    import sys
    import types

    try:
        from antenv.axon_hooks import get_axon_ntff_profile_hook  # noqa: F401

        return
    except ImportError:
        pass
    try:
        from trn_agent_boot.trn_boot import _ntff_profile_via_ctypes

        hook = _ntff_profile_via_ctypes("/opt/axon/libaxon_pjrt.so")
    except Exception:
        hook = None
    mod = types.ModuleType("antenv.axon_hooks")
    mod.get_axon_ntff_profile_hook = lambda: hook
    sys.modules["antenv.axon_hooks"] = mod


def kernel(x, W_gate, b_gate, _trace=False):
    global LAST_EXEC_NS, LAST_TRACE
    import os

    from concourse.bass_utils import run_bass_kernel_spmd

    if _trace or os.environ.get("BASS_TRACE"):
        _ensure_ntff_shim()

    x = np.ascontiguousarray(np.asarray(x, dtype=np.float32))
    wt = np.asarray(W_gate, dtype=np.float32).T  # [NB, D]
    negb = -np.asarray(b_gate, dtype=np.float32).reshape(N_BRANCHES, 1)
    w = np.ascontiguousarray(np.concatenate([wt, negb], axis=1))  # [NB, D+1]

    nc = _get_nc()
    in_maps = [
        {"x": x[c * N_SHARD : (c + 1) * N_SHARD], "w": w}
        for c in range(N_CORES)
    ]
    res = run_bass_kernel_spmd(
        nc, in_maps, core_ids=list(range(N_CORES)), trace=_trace
    )
    LAST_EXEC_NS = res.exec_time_ns
    LAST_TRACE = getattr(res, "instructions_and_trace", None)

    x0 = np.concatenate([res.results[c]["o0"] for c in range(N_CORES)], axis=0)
    x1 = np.concatenate([res.results[c]["o1"] for c in range(N_CORES)], axis=0)
    xc = np.concatenate([res.results[c]["oc"] for c in range(N_CORES)], axis=0)
    return (x0, x1, xc)


# revision 40
# speedup vs baseline: 1.0312x; 1.0312x over previous
"""Trainium2 Bass kernel for nn_BranchRoute (threshold MoE routing).

reference:
    score = sigmoid(x @ W_gate + b_gate)          # [N, 2]
    hot   = score > 0.5                           # == (x @ W_gate + b_gate) > 0
    x_0   = where(hot[:, 0:1], x, 0)
    x_1   = where(hot[:, 1:2], x, 0)
    x_comb = x_0 + x_1

Sharding: data-parallel over tokens across 8 NeuronCores (2048 tokens/core),
gate weights replicated.  Per core the kernel streams 16 tiles of
[128 tokens, 1024 d]: gate logits via fused multiply+reduce on DVE
(sigmoid(z) > 0.5  <=>  z > -b, so no sigmoid is evaluated), then three
per-partition-scalar mask multiplies (x0/x1 on ACT, x_comb on DVE), and
per-tile stores split across both HWDGE queues while loads prefetch on the
Pool SWDGE queue.  The kernel is DMA-bound: 8 MiB in + 24 MiB out per core
~= 93 us at the measured ~368 GB/s per-core HBM rate; it executes in
~99-105 us.
"""

import numpy as np

N_TOKENS = 16384
D_MODEL = 1024
N_BRANCHES = 2
N_CORES = 8
N_SHARD = N_TOKENS // N_CORES  # 2048 tokens per core
P = 128                        # SBUF partitions
NTILES = N_SHARD // P          # 16 token-tiles per core

_CACHE = {}


def _split_multi_waits(nc, max_embedded=1):
    """This container's walrus build rejects instructions carrying more than
    one embedded semaphore wait ("Too many sync wait commands").  Hoist the
    extra waits into standalone EventSemaphore instructions immediately
    before the owning instruction on the same engine — identical ordering
    semantics, encodable by this compiler."""
    from concourse import mybir

    wid = 0
    for fn in nc.m.functions:
        for bb in fn.blocks:
            out = []
            changed = False
            for inst in bb.instructions:
                si = getattr(inst, "sync_info", None)
                waits = list(si.on_wait) if si is not None else []
                if si is not None and len(waits) > max_embedded:
                    extra, keep = waits[:-max_embedded], waits[-max_embedded:]
                    for w in extra:
                        es = mybir.InstEventSemaphore(
                            name=f"WSPLIT-{wid}", ins=[], outs=[]
                        )
                        wid += 1
                        es.engine = inst.engine
                        es.sync_info = mybir.SyncInfo(on_wait=[w], on_update=[])
                        out.append(es)
                    si.on_wait = keep
                    changed = True
                out.append(inst)
            if changed:
                bb.instructions = out


def _build_bass(dma_cfg="C"):
    import concourse.bass as bass
    import concourse.tile as tile
    from concourse import mybir

    f32 = mybir.dt.float32
    nc = bass.Bass(trn_type="TRN2", num_swdge_queues=2)

    # w is passed host-side as [N_BRANCHES, D_MODEL + 1]: row br holds
    # W[:, br] transposed (contiguous, so the partition-broadcast DMA reads
    # 4 KiB bursts) with -b[br] appended as the last column.
    DW = D_MODEL + 1
    x_h = nc.dram_tensor("x", [N_SHARD, D_MODEL], f32, kind="ExternalInput")
    w_h = nc.dram_tensor("w", [N_BRANCHES, DW], f32, kind="ExternalInput")
    o0_h = nc.dram_tensor("o0", [N_SHARD, D_MODEL], f32, kind="ExternalOutput")
    o1_h = nc.dram_tensor("o1", [N_SHARD, D_MODEL], f32, kind="ExternalOutput")
    oc_h = nc.dram_tensor("oc", [N_SHARD, D_MODEL], f32, kind="ExternalOutput")

    # Pair token-tiles: [npair, 128, 2, 1024] — one 1 MiB DMA per pair,
    # partition dim leading on both sides so the DMA APs balance.
    TB = 2
    NPAIR = NTILES // TB
    x_t = x_h[:].rearrange("(t s p) d -> t p s d", s=TB, p=P)
    o0_t = o0_h[:].rearrange("(t s p) d -> t p s d", s=TB, p=P)
    o1_t = o1_h[:].rearrange("(t s p) d -> t p s d", s=TB, p=P)
    oc_t = oc_h[:].rearrange("(t s p) d -> t p s d", s=TB, p=P)

    with tile.TileContext(nc) as tc:
        with (
            tc.tile_pool(name="singles", bufs=1) as singles,
            tc.tile_pool(name="xp", bufs=6) as xp,
            tc.tile_pool(name="scr", bufs=3) as scr,
            tc.tile_pool(name="out0", bufs=6) as p0,
            tc.tile_pool(name="out1", bufs=6) as p1,
            tc.tile_pool(name="outc", bufs=6) as pc,
            tc.tile_pool(name="small", bufs=8) as small,
        ):
            # [W^T | -b] rows broadcast across all 128 partitions.  A single
            # step-0-partition DRAM broadcast DMA measures ~10us and stalls
            # startup, so split it into 4 concurrent 32-partition chunks on
            # the ACT HWDGE queue (the SP queue carries the first x load).
            # (A serial on-chip doubling chain was measured slower: each
            # chained SBUF->SBUF DMA pays issue+sem latency.)
            # wb[p, br*DW : br*DW+D] = W[:, br],  wb[p, br*DW+D] = -b[br]
            wb = singles.tile([P, N_BRANCHES * DW], f32)
            w_ap = w_h[:]
            PCHUNK = 32
            for ci in range(P // PCHUNK):
                w_bcast = bass.AP(
                    tensor=w_ap.tensor,
                    offset=w_ap.offset,
                    ap=[[0, PCHUNK], [1, N_BRANCHES * DW]],
                )
                nc.scalar.dma_start(
                    out=wb[ci * PCHUNK : (ci + 1) * PCHUNK, :], in_=w_bcast
                )
            # negb[p, br] = -b[br] as a strided view of wb
            negb = bass.AP(
                tensor=wb.tensor,
                offset=wb.offset + D_MODEL,
                ap=[wb.ap[0], [DW, N_BRANCHES]],
            )

            for i in range(NPAIR):
                # dma_cfg picks the load queue: A = ACT HWDGE, B = SP HWDGE,
                # C = Pool SWDGE (prefetched ahead, so SWDGE latency hides)
                # except the critical first pair, which takes the fast SP
                # HWDGE path to cut the startup stall.
                x_sb = xp.tile([P, TB, D_MODEL], f32)
                ld = {
                    "A": nc.scalar,
                    "B": nc.sync,
                    "C": nc.sync if i == 0 else nc.gpsimd,
                }[dma_cfg]
                ld.dma_start(out=x_sb, in_=x_t[i])

                op0_pair = op1_pair = opc_pair = None
                if dma_cfg in ("A", "B"):
                    op0_pair = p0.tile([P, TB, D_MODEL], f32, tag="o0pair")
                    op1_pair = p1.tile([P, TB, D_MODEL], f32, tag="o1pair")
                    opc_pair = pc.tile([P, TB, D_MODEL], f32, tag="ocpair")

                for s in range(TB):
                    x_s = x_sb[:, s, :]

                    # z[p, br] = sum_d x[p, d] * W[d, br]  (fused DVE pass/branch)
                    z = small.tile([P, N_BRANCHES], f32)
                    for br in range(N_BRANCHES):
                        scratch = scr.tile([P, D_MODEL], f32)
                        nc.vector.scalar_tensor_tensor(
                            out=scratch,
                            in0=x_s,
                            scalar=0.0,
                            in1=wb[:, br * DW : br * DW + D_MODEL],
                            op0=mybir.AluOpType.bypass,
                            op1=mybir.AluOpType.mult,
                            accum_out=z[:, br : br + 1],
                        )

                    # hot mask: m = (z > -b) as 1.0/0.0 ; mc = m0 + m1
                    m = small.tile([P, N_BRANCHES], f32)
                    nc.vector.tensor_tensor(
                        out=m, in0=z, in1=negb, op=mybir.AluOpType.is_gt
                    )
                    mc = small.tile([P, 1], f32)
                    nc.vector.tensor_add(out=mc, in0=m[:, 0:1], in1=m[:, 1:2])

                    # masked outputs: x * m (per-partition scalar broadcast)
                    if dma_cfg == "C":
                        o0 = p0.tile([P, D_MODEL], f32)
                        o1 = p1.tile([P, D_MODEL], f32)
                        oc = pc.tile([P, D_MODEL], f32)
                    else:
                        o0 = op0_pair[:, s, :]
                        o1 = op1_pair[:, s, :]
                        oc = opc_pair[:, s, :]
                    nc.scalar.mul(out=o0, in_=x_s, mul=m[:, 0:1])
                    nc.scalar.mul(out=o1, in_=x_s, mul=m[:, 1:2])
                    nc.vector.tensor_scalar_mul(out=oc, in0=x_s, scalar1=mc)

                    if dma_cfg == "C":
                        # Per-sub-tile stores split across both HWDGE queues
                        # (12 MiB per queue).
                        qa = nc.sync if (i * TB + s) % 2 == 0 else nc.scalar
                        nc.sync.dma_start(out=o0_t[i][:, s, :], in_=o0)
                        nc.scalar.dma_start(out=o1_t[i][:, s, :], in_=o1)
                        qa.dma_start(out=oc_t[i][:, s, :], in_=oc)

                if dma_cfg == "A":
                    # Loads on ACT queue; all stores on the SP queue.
                    nc.sync.dma_start(out=o0_t[i], in_=op0_pair)
                    nc.sync.dma_start(out=o1_t[i], in_=op1_pair)
                    nc.sync.dma_start(out=oc_t[i], in_=opc_pair)
                elif dma_cfg == "B":
                    # Loads + oc on SP queue; o0/o1 on the ACT queue.
                    nc.scalar.dma_start(out=o0_t[i], in_=op0_pair)
                    nc.scalar.dma_start(out=o1_t[i], in_=op1_pair)
                    nc.sync.dma_start(out=oc_t[i], in_=opc_pair)

    _split_multi_waits(nc)
    return nc


def _get_nc():
    if "nc" not in _CACHE:
        _CACHE["nc"] = _build_bass()
    return _CACHE["nc"]


LAST_EXEC_NS = None
LAST_TRACE = None


def _ensure_ntff_shim():
    """antenv.axon_hooks is absent in this container image; when tracing is
    active (trace=True or BASS_TRACE set) run_bass_kernel_spmd imports it.
    Recreate it from the ctypes implementation shipped in trn_agent_boot."""
    import sys
    import types

    try:
        from antenv.axon_hooks import get_axon_ntff_profile_hook  # noqa: F401

        return
    except ImportError:
        pass
    try:
        from trn_agent_boot.trn_boot import _ntff_profile_via_ctypes

        hook = _ntff_profile_via_ctypes("/opt/axon/libaxon_pjrt.so")
    except Exception:
        hook = None
    mod = types.ModuleType("antenv.axon_hooks")
    mod.get_axon_ntff_profile_hook = lambda: hook
    sys.modules["antenv.axon_hooks"] = mod


def kernel(x, W_gate, b_gate, _trace=False):
    global LAST_EXEC_NS, LAST_TRACE
    import os

    from concourse.bass_utils import run_bass_kernel_spmd

    if _trace or os.environ.get("BASS_TRACE"):
        _ensure_ntff_shim()

    x = np.ascontiguousarray(np.asarray(x, dtype=np.float32))
    wt = np.asarray(W_gate, dtype=np.float32).T  # [NB, D]
    negb = -np.asarray(b_gate, dtype=np.float32).reshape(N_BRANCHES, 1)
    w = np.ascontiguousarray(np.concatenate([wt, negb], axis=1))  # [NB, D+1]

    nc = _get_nc()
    in_maps = [
        {"x": x[c * N_SHARD : (c + 1) * N_SHARD], "w": w}
        for c in range(N_CORES)
    ]
    res = run_bass_kernel_spmd(
        nc, in_maps, core_ids=list(range(N_CORES)), trace=_trace
    )
    LAST_EXEC_NS = res.exec_time_ns
    LAST_TRACE = getattr(res, "instructions_and_trace", None)

    x0 = np.concatenate([res.results[c]["o0"] for c in range(N_CORES)], axis=0)
    x1 = np.concatenate([res.results[c]["o1"] for c in range(N_CORES)], axis=0)
    xc = np.concatenate([res.results[c]["oc"] for c in range(N_CORES)], axis=0)
    return (x0, x1, xc)
